# revision 1
# baseline (speedup 1.0000x reference)
"""DeepSet/GNN message-passing layer on 8 Trainium2 NeuronCores (Bass/Tile).

Math (reference):
    msg_sum = segment_sum(x[src], dst);  counts = hist(dst)
    mean    = msg_sum / max(counts, 1)
    out     = x@W1 + b1 + (x - mean)@W2 + b2,  except rows with counts==0 keep x.

Rewritten:
    out = x @ (W1+W2) + (b1+b2) - mean @ W2
    mean[i] = sum_{e: dst_e=i} x[src_e] / counts[i]

Device strategy (per core, SPMD over 8 cores):
  * Nodes are packed into 392 tiles of 128 (snake-deal by in-degree so each
    tile has ~1020 incoming edges), tiles snake-dealt to 8 cores (49 each).
  * Edges are routed host-side to (core, tile, chunk-of-128) slots; a chunk's
    indicator matrix S'[e, n] = (dstloc[e] == n) * (1/count[dst_e]) is built
    on VectorE from an iota constant; then TensorE computes
       meanT[din, n] += G_chunk[:, din_c].T @ S'   (G = gathered x[src] rows)
    so the segment-mean lands directly in [din, nodes] (lhsT) layout.
  * One PSUM bank accumulates the full output tile:
       out_psum = sum_c xT_c.T @ W12_c + ones.T @ b12 + sum_c meanT_c.T @ (-W2)_c
  * x is replicated on every core as a bf16 gather table; x^T tiles, weights
    and edge-routing arrays are uploaded per-core.
  * Host applies the counts==0 passthrough fix-up (a handful of rows).
"""

import numpy as np
import ml_dtypes

N_NODES = 50000
D = 512
N_CORES = 8
P = 128
NT_TOT = 392           # node tiles total (392*128 = 50176 >= 50000)
TPC = NT_TOT // N_CORES  # 49 tiles per core
NPAD = NT_TOT * P
DC = D // P            # 4 contraction chunks of 128
H = 25088              # x_table split point (dma_gather uses int16 indices)


def _route(src, dst, counts):
    """Host-side routing: node->tile packing, tile->core deal, edge->chunk-slot
    layout. Returns per-core arrays + the uniform per-slot chunk schedule."""
    cpad = np.zeros(NPAD, np.int64)
    cpad[:N_NODES] = counts

    # --- nodes -> tiles: snake-deal in descending-degree order ---
    order = np.argsort(-cpad, kind="stable")
    tile_of_node = np.empty(NPAD, np.int32)
    slot_of_node = np.empty(NPAD, np.int32)
    fwd = np.arange(NT_TOT, dtype=np.int32)
    for r in range(P):
        ids = order[r * NT_TOT:(r + 1) * NT_TOT]
        tiles = fwd if (r % 2 == 0) else fwd[::-1]
        tile_of_node[ids] = tiles
        slot_of_node[ids] = r

    tile_sums = np.zeros(NT_TOT, np.int64)
    np.add.at(tile_sums, tile_of_node[:N_NODES], counts)

    # --- tiles -> cores: snake-deal in descending-edges order ---
    t_order = np.argsort(-tile_sums, kind="stable")
    core_of_tile = np.empty(NT_TOT, np.int32)
    cslot_of_tile = np.empty(NT_TOT, np.int32)  # per-core tile slot 0..TPC-1
    fwd8 = np.arange(N_CORES, dtype=np.int32)
    for r in range(TPC):
        ids = t_order[r * N_CORES:(r + 1) * N_CORES]
        cores = fwd8 if (r % 2 == 0) else fwd8[::-1]
        core_of_tile[ids] = cores
        cslot_of_tile[ids] = r

    # edges per (core, slot, table-half): src < H goes to half 0
    e_tile = tile_of_node[dst]
    e_core = core_of_tile[e_tile].astype(np.int64)
    e_cslot = cslot_of_tile[e_tile].astype(np.int64)
    e_half = (src >= H).astype(np.int64)
    ecnt = np.zeros((N_CORES, TPC, 2), np.int64)
    np.add.at(ecnt, (e_core, e_cslot, e_half), 1)

    # uniform per-slot chunk schedule (max over cores), per table half
    KH = -(-ecnt.max(axis=0) // P)   # [TPC, 2] ceil div
    K = KH.sum(axis=1)               # combined chunks per slot
    g0 = np.concatenate([[0], np.cumsum(K)])
    CT = int(g0[-1])

    # --- per-core edge arrays laid out [P, CT] (partition = pos in chunk) ---
    esrc = np.zeros((N_CORES, P, CT), np.int32)
    edst = np.full((N_CORES, P, CT), -1.0, np.float32)
    erec = np.zeros((N_CORES, P, CT), np.float32)
    # int16 gather indices, wrapped [j%16, j//16] per gather block and
    # replicated over partition groups of 16 (dma_gather's index layout)
    eidx = np.zeros((N_CORES, P, 8 * CT), np.int16)

    ekey = (e_core * TPC + e_cslot) * 2 + e_half
    eorder = np.argsort(ekey, kind="stable")
    s_src = src[eorder]
    s_dst = dst[eorder]
    s_key = ekey[eorder]
    recip_all = 1.0 / np.maximum(cpad, 1).astype(np.float32)
    bounds = np.searchsorted(s_key, np.arange(N_CORES * TPC * 2 + 1))
    for c in range(N_CORES):
        for j in range(TPC):
            for h in range(2):
                key = (c * TPC + j) * 2 + h
                lo, hi = bounds[key], bounds[key + 1]
                n = hi - lo
                kh = int(KH[j, h])
                base = int(g0[j]) + (0 if h == 0 else int(KH[j, 0]))
                if n:
                    ss = s_src[lo:hi]
                    sd = s_dst[lo:hi]
                    pos = np.arange(n)
                    pp = pos % P
                    gg = base + pos // P
                    esrc[c, pp, gg] = ss
                    edst[c, pp, gg] = slot_of_node[sd].astype(np.float32)
                    erec[c, pp, gg] = recip_all[sd]
                if kh:
                    blk = np.zeros((16, kh * 8), np.int16)
                    if n:
                        val = (ss if h == 0 else ss - H).astype(np.int16)
                        blk[pos % 16, pos // 16] = val
                    eidx[c, :, 8 * base:8 * (base + kh)] = np.tile(blk, (8, 1))

    # node id for (core, tileslot, nodeslot) — for xT layout + output unshard
    node_at = np.empty((N_CORES, TPC, P), np.int64)
    node_ids = np.arange(NPAD)
    flat_idx = (core_of_tile[tile_of_node].astype(np.int64) * TPC * P
                + cslot_of_tile[tile_of_node].astype(np.int64) * P
                + slot_of_node)
    node_at.reshape(-1)[flat_idx] = node_ids
    return esrc, edst, erec, eidx, node_at, K, KH, g0, CT


def _build_program(K, KH, g0, CT, repeats=1, opts=None):
    opts = opts or {}
    import concourse.bacc as bacc
    import concourse.bass as bass
    import concourse.tile as tile
    import concourse.mybir as mybir

    f32 = mybir.dt.float32
    bf16 = mybir.dt.bfloat16
    i32 = mybir.dt.int32
    i16 = mybir.dt.int16

    nc = bacc.Bacc("TRN2", target_bir_lowering=False, debug=False,
                   num_devices=N_CORES,
                   num_swdge_queues=opts.get("nq", 1))

    x_table = nc.dram_tensor("x_table", [N_NODES, D], bf16, kind="ExternalInput")
    xTl = nc.dram_tensor("xTl", [P, TPC * D], bf16, kind="ExternalInput")
    w12l = nc.dram_tensor("w12l", [P, DC * D], bf16, kind="ExternalInput")
    w2nl = nc.dram_tensor("w2nl", [P, DC * D], bf16, kind="ExternalInput")
    b12 = nc.dram_tensor("b12", [1, D], bf16, kind="ExternalInput")
    esrc = nc.dram_tensor("esrc", [P, CT], i32, kind="ExternalInput")
    edst = nc.dram_tensor("edst", [P, CT], f32, kind="ExternalInput")
    erec = nc.dram_tensor("erec", [P, CT], f32, kind="ExternalInput")
    eidx = nc.dram_tensor("eidx", [P, 8 * CT], i16, kind="ExternalInput")
    iota_in = nc.dram_tensor("iota_in", [P, P], f32, kind="ExternalInput")
    ident_in = nc.dram_tensor("ident_in", [P, P], f32, kind="ExternalInput")
    if opts.get("hosty"):
        y_table = nc.dram_tensor("y_table", [N_NODES, D], bf16,
                                 kind="ExternalInput")
    out = nc.dram_tensor("out", [TPC * P, D], f32, kind="ExternalOutput")

    with tile.TileContext(nc) as tc:
        with (
            tc.tile_pool(name="res", bufs=1) as res,
            tc.tile_pool(name="gpool", bufs=opts.get("g_bufs", 3)) as gpool,
            tc.tile_pool(name="spool", bufs=int(K.max()) + 4) as spool,
            tc.tile_pool(name="mpool", bufs=3) as mpool,
            tc.tile_pool(name="opool", bufs=3) as opool,
            tc.tile_pool(name="pmean", bufs=opts.get("pmean_bufs", 2),
                         space="PSUM") as pmean,
            tc.tile_pool(name="ptrp", bufs=2, space="PSUM") as ptrp,
            tc.tile_pool(name="pout", bufs=opts.get("pout_bufs", 2),
                         space="PSUM") as pout,
        ):
            # resident constants
            xTl_sb = res.tile([P, TPC * D], bf16)
            nc.sync.dma_start(out=xTl_sb[:], in_=xTl[:])
            w12_sb = res.tile([P, DC * D], bf16)
            nc.sync.dma_start(out=w12_sb[:], in_=w12l[:])
            w2n_sb = res.tile([P, DC * D], bf16)
            nc.sync.dma_start(out=w2n_sb[:], in_=w2nl[:])
            b12_sb = res.tile([1, D], bf16)
            nc.sync.dma_start(out=b12_sb[:], in_=b12[:])
            esrc_sb = res.tile([P, CT], i32)
            nc.sync.dma_start(out=esrc_sb[:], in_=esrc[:])
            edst_sb = res.tile([P, CT], f32)
            nc.sync.dma_start(out=edst_sb[:], in_=edst[:])
            erec_sb = res.tile([P, CT], f32)
            nc.sync.dma_start(out=erec_sb[:], in_=erec[:])
            eidx_sb = res.tile([P, 8 * CT], i16)
            nc.sync.dma_start(out=eidx_sb[:], in_=eidx[:])
            iota_sb = res.tile([P, P], f32)
            nc.sync.dma_start(out=iota_sb[:], in_=iota_in[:])
            ident_sb = res.tile([P, P], f32)
            nc.sync.dma_start(out=ident_sb[:], in_=ident_in[:])
            ones_sb = res.tile([1, P], bf16)
            nc.vector.memset(ones_sb[:], 1.0)

            # software-pipelined: emit segment-phase(t), then dense-phase(t-1)
            pending = None  # (meanT_sb tile, tile idx)
            rep_tiles = [t for _ in range(repeats) for t in range(TPC)]

            def emit_gather(G, t, kt, gbase, table):
                k0, k1 = int(KH[t, 0]), int(KH[t, 1])
                for h, kh, coff in ((0, k0, 0), (1, k1, k0)):
                    if kh == 0:
                        continue
                    tbl = table[0:H, :] if h == 0 else table[H:N_NODES, :]
                    nc.gpsimd.dma_gather(
                        out_ap=G[:, coff * D:(coff + kh) * D].rearrange(
                            "p (k d) -> p k d", d=D),
                        in_ap=tbl,
                        idxs_ap=eidx_sb[:, 8 * (gbase + coff):
                                        8 * (gbase + coff + kh)],
                        num_idxs=kh * P,
                        num_idxs_reg=kh * P,
                        elem_size=D,
                        queue_num=(2 * t + h) % opts.get("nq", 1))

            def emit_S(gidx):
                S = spool.tile([P, P], bf16)
                nc.vector.tensor_scalar(
                    out=S[:], in0=iota_sb[:],
                    scalar1=edst_sb[:, gidx:gidx + 1],
                    scalar2=erec_sb[:, gidx:gidx + 1],
                    op0=mybir.AluOpType.is_equal,
                    op1=mybir.AluOpType.mult)
                return S

            if opts.get("hosty"):
                # single-phase: segment matmuls accumulate -mean@W2 directly
                # into the output PSUM from gathered y=x@(-W2) rows, then the
                # dense x@W12 + b12 matmuls extend the same group.
                for t in rep_tiles:
                    kt = int(K[t])
                    gbase = int(g0[t])
                    G = gpool.tile([P, kt * D], bf16, tag="G")
                    emit_gather(G, t, kt, gbase, y_table)
                    Ss = [emit_S(gbase + g) for g in range(kt)]
                    po = pout.tile([P, D], f32)
                    for g in range(kt):
                        nc.tensor.matmul(
                            out=po[:], lhsT=Ss[g][:],
                            rhs=G[:, g * D:(g + 1) * D],
                            start=(g == 0), stop=False)
                    for c in range(DC):
                        nc.tensor.matmul(
                            out=po[:],
                            lhsT=xTl_sb[:, (t * DC + c) * P:
                                        (t * DC + c + 1) * P],
                            rhs=w12_sb[:, c * D:(c + 1) * D],
                            start=False, stop=False)
                    nc.tensor.matmul(out=po[:], lhsT=ones_sb[:, :],
                                     rhs=b12_sb[:, :], start=False, stop=True)
                    out_sb = opool.tile([P, D], f32)
                    nc.vector.tensor_copy(out=out_sb[:], in_=po[:])
                    if not opts.get("no_store"):
                        nc.sync.dma_start(out=out[t * P:(t + 1) * P, :],
                                          in_=out_sb[:])
                rep_tiles = []

            def dense_phase(meanT_sb, t):
                po = pout.tile([P, D], f32)
                for c in range(DC):
                    nc.tensor.matmul(
                        out=po[:],
                        lhsT=xTl_sb[:, (t * DC + c) * P:(t * DC + c + 1) * P],
                        rhs=w12_sb[:, c * D:(c + 1) * D],
                        start=(c == 0), stop=False)
                nc.tensor.matmul(out=po[:], lhsT=ones_sb[:, :],
                                 rhs=b12_sb[:, :], start=False, stop=False)
                for c in range(DC):
                    nc.tensor.matmul(
                        out=po[:],
                        lhsT=meanT_sb[:, c * P:(c + 1) * P],
                        rhs=w2n_sb[:, c * D:(c + 1) * D],
                        start=False, stop=(c == DC - 1))
                out_sb = opool.tile([P, D], f32)
                nc.vector.tensor_copy(out=out_sb[:], in_=po[:])
                if not opts.get("no_store"):
                    nc.sync.dma_start(out=out[t * P:(t + 1) * P, :],
                                      in_=out_sb[:])

            for t in rep_tiles:
                kt = int(K[t])
                gbase = int(g0[t])
                pm = (None if opts.get("no_seg")
                      else pmean.tile([P, D], f32))
                # gather x[src] rows for this tile's edges via the ext-isa
                # dma_gather (one instruction per table half; indices are
                # int16 so the 50k-row table is split at H).
                # NOTE: batching an indirect_dma_start with a [128, kt]
                # offset AP works in CoreSim but mis-lowers on HW — use
                # dma_gather or per-chunk [128, 1] indirect DMAs only.
                G = (None if opts.get("no_seg")
                     else gpool.tile([P, kt * D], bf16, tag="G"))
                if opts.get("no_seg"):
                    pass
                elif opts.get("no_gather"):
                    nc.vector.memset(G[:], 0.0)
                elif opts.get("indirect"):
                    for g in range(kt):
                        gidx = gbase + g
                        nc.gpsimd.indirect_dma_start(
                            out=G[:, g * D:(g + 1) * D], out_offset=None,
                            in_=x_table[:],
                            in_offset=bass.IndirectOffsetOnAxis(
                                ap=esrc_sb[:, gidx:gidx + 1], axis=0))
                else:
                    k0, k1 = int(KH[t, 0]), int(KH[t, 1])
                    for h, kh, coff in ((0, k0, 0), (1, k1, k0)):
                        if kh == 0:
                            continue
                        tbl = x_table[0:H, :] if h == 0 else x_table[H:N_NODES, :]
                        nc.gpsimd.dma_gather(
                            out_ap=G[:, coff * D:(coff + kh) * D].rearrange(
                                "p (k d) -> p k d", d=D),
                            in_ap=tbl,
                            idxs_ap=eidx_sb[:, 8 * (gbase + coff):
                                            8 * (gbase + coff + kh)],
                            num_idxs=kh * P,
                            num_idxs_reg=kh * P,
                            elem_size=D,
                            queue_num=(2 * t + h) % opts.get("nq", 1))
                meanT_sb = mpool.tile([P, D], bf16, tag="meanT")
                if opts.get("no_seg"):
                    nc.vector.memset(meanT_sb[:], 0.0)
                else:
                    Ss = []
                    for g in range(kt):
                        gidx = gbase + g
                        S = spool.tile([P, P], bf16)
                        nc.vector.tensor_scalar(
                            out=S[:], in0=iota_sb[:],
                            scalar1=edst_sb[:, gidx:gidx + 1],
                            scalar2=erec_sb[:, gidx:gidx + 1],
                            op0=mybir.AluOpType.is_equal,
                            op1=mybir.AluOpType.mult)
                        Ss.append(S)
                    if opts.get("segC"):
                        # mean in [node, din]: one wide matmul per chunk
                        # (stationary S reused for all 512 moving cols), then
                        # transpose via 4 SBUF->SBUF DMA transposes.
                        # (PE identity transposes are broken here: bf16 PSUM
                        # gives wrong data, f32 wedges the exec unit.)
                        for g in range(kt):
                            nc.tensor.matmul(
                                out=pm[:],
                                lhsT=Ss[g][:],
                                rhs=G[:, g * D:(g + 1) * D],
                                start=(g == 0), stop=(g == kt - 1))
                        mean_sb = mpool.tile([P, D], bf16, tag="mean_bf")
                        nc.scalar.activation(
                            out=mean_sb[:], in_=pm[:],
                            func=mybir.ActivationFunctionType.Copy)
                        for c in range(DC):
                            nc.sync.dma_start_transpose(
                                out=meanT_sb[:, c * P:(c + 1) * P],
                                in_=mean_sb[:, c * P:(c + 1) * P])
                    else:
                        # meanT accumulation [din, node]: one PSUM
                        # accumulation group per 128-col slice (groups in the
                        # same bank must not interleave)
                        for c in range(DC):
                            for g in range(kt):
                                nc.tensor.matmul(
                                    out=pm[:, c * P:(c + 1) * P],
                                    lhsT=G[:, g * D + c * P:g * D + (c + 1) * P],
                                    rhs=Ss[g][:],
                                    start=(g == 0), stop=(g == kt - 1))
                        nc.scalar.activation(
                            out=meanT_sb[:], in_=pm[:],
                            func=mybir.ActivationFunctionType.Copy)
                if pending is not None:
                    dense_phase(*pending)
                pending = (meanT_sb, t)
            if pending is not None:
                dense_phase(*pending)

    nc.compile()
    return nc


def _pack(x, src, dst, W1, b1, W2, b2):
    counts = np.bincount(dst, minlength=N_NODES)
    esrc, edst, erec, eidx, node_at, K, KH, g0, CT = _route(src, dst, counts)

    x_pad = np.zeros((NPAD, D), np.float32)
    x_pad[:N_NODES] = x
    bf = ml_dtypes.bfloat16
    x_table = x.astype(bf)
    y_table = (x @ (-W2)).astype(bf)   # for the hosty variant

    W12 = (W1 + W2).astype(np.float32)
    W2n = (-W2).astype(np.float32)
    # w layout: [:, c*D:(c+1)*D] = W[c*128:(c+1)*128, :]
    w12l = np.ascontiguousarray(
        W12.reshape(DC, P, D).transpose(1, 0, 2).reshape(P, DC * D)).astype(bf)
    w2nl = np.ascontiguousarray(
        W2n.reshape(DC, P, D).transpose(1, 0, 2).reshape(P, DC * D)).astype(bf)
    b12 = (b1 + b2).astype(np.float32).reshape(1, D).astype(bf)

    in_maps = []
    for c in range(N_CORES):
        xo = x_pad[node_at[c].reshape(-1)]                    # [TPC*P, D]
        # xTl[p, (t*DC+cc)*P + n] = xo[t*P+n, cc*P+p]
        xTl = np.ascontiguousarray(
            xo.reshape(TPC, P, DC, P).transpose(3, 0, 2, 1).reshape(P, TPC * D)
        ).astype(bf)
        in_maps.append({
            "x_table": x_table,
            "y_table": y_table,
            "xTl": xTl,
            "w12l": w12l,
            "w2nl": w2nl,
            "b12": b12,
            "esrc": np.ascontiguousarray(esrc[c]),
            "edst": np.ascontiguousarray(edst[c]),
            "erec": np.ascontiguousarray(erec[c]),
            "eidx": np.ascontiguousarray(eidx[c]),
            "iota_in": np.tile(np.arange(P, dtype=np.float32), (P, 1)),
            "ident_in": np.eye(P, dtype=np.float32),
        })
    return in_maps, node_at, counts, K, KH, g0, CT


def _unshard(results, node_at, counts, x):
    out_full = np.empty((NPAD, D), np.float32)
    for c in range(N_CORES):
        out_full[node_at[c].reshape(-1)] = results[c]["out"]
    out_full = out_full[:N_NODES]
    zero = counts == 0
    out_full[zero] = x[zero]
    return out_full


def pack_from_inputs(inp):
    return _pack(np.asarray(inp["x"], np.float32),
                 np.asarray(inp["src"]).astype(np.int64),
                 np.asarray(inp["dst"]).astype(np.int64),
                 np.asarray(inp["W1"], np.float32),
                 np.asarray(inp["b1"], np.float32),
                 np.asarray(inp["W2"], np.float32),
                 np.asarray(inp["b2"], np.float32))


def kernel(**inputs):
    x = np.asarray(inputs["x"], np.float32)
    src = np.asarray(inputs["src"]).astype(np.int64)
    dst = np.asarray(inputs["dst"]).astype(np.int64)
    W1 = np.asarray(inputs["W1"], np.float32)
    b1 = np.asarray(inputs["b1"], np.float32)
    W2 = np.asarray(inputs["W2"], np.float32)
    b2 = np.asarray(inputs["b2"], np.float32)

    in_maps, node_at, counts, K, KH, g0, CT = _pack(x, src, dst, W1, b1, W2, b2)
    nc = _build_program(K, KH, g0, CT)

    from concourse.bass_utils import run_bass_kernel_spmd
    res = run_bass_kernel_spmd(nc, in_maps, core_ids=list(range(N_CORES)))
    return _unshard(res.results, node_at, counts, x)



# revision 2
# speedup vs baseline: 1.4459x; 1.4459x over previous
"""DeepSet/GNN message-passing layer on 8 Trainium2 NeuronCores (Bass/Tile).

Math (reference):
    msg_sum = segment_sum(x[src], dst);  counts = hist(dst)
    mean    = msg_sum / max(counts, 1)
    out     = x@W1 + b1 + (x - mean)@W2 + b2,  except rows with counts==0 keep x.

Rewritten:
    out = x @ (W1+W2) + (b1+b2) - mean @ W2
    mean[i] = sum_{e: dst_e=i} x[src_e] / counts[i]

Device strategy (per core, SPMD over 8 cores):
  * Nodes are packed into 392 tiles of 128 (snake-deal by in-degree so each
    tile has ~1020 incoming edges), tiles snake-dealt to 8 cores (49 each).
  * Edges are routed host-side to (core, tile, chunk-of-128) slots; a chunk's
    indicator matrix S'[e, n] = (dstloc[e] == n) * (1/count[dst_e]) is built
    on VectorE from an iota constant; then TensorE computes
       meanT[din, n] += G_chunk[:, din_c].T @ S'   (G = gathered x[src] rows)
    so the segment-mean lands directly in [din, nodes] (lhsT) layout.
  * One PSUM bank accumulates the full output tile:
       out_psum = sum_c xT_c.T @ W12_c + ones.T @ b12 + sum_c meanT_c.T @ (-W2)_c
  * x is replicated on every core as a bf16 gather table; x^T tiles, weights
    and edge-routing arrays are uploaded per-core.
  * Host applies the counts==0 passthrough fix-up (a handful of rows).
"""

import numpy as np
import ml_dtypes

N_NODES = 50000
D = 512
N_CORES = 8
P = 128
NT_TOT = 392           # node tiles total (392*128 = 50176 >= 50000)
TPC = NT_TOT // N_CORES  # 49 tiles per core
NPAD = NT_TOT * P
DC = D // P            # 4 contraction chunks of 128
H = 25088              # x_table split point (dma_gather uses int16 indices)


def _route(src, dst, counts):
    """Host-side routing: node->tile packing, tile->core deal, edge->chunk-slot
    layout. Returns per-core arrays + the uniform per-slot chunk schedule."""
    cpad = np.zeros(NPAD, np.int64)
    cpad[:N_NODES] = counts

    # --- nodes -> tiles: snake-deal in descending-degree order ---
    order = np.argsort(-cpad, kind="stable")
    tile_of_node = np.empty(NPAD, np.int32)
    slot_of_node = np.empty(NPAD, np.int32)
    fwd = np.arange(NT_TOT, dtype=np.int32)
    for r in range(P):
        ids = order[r * NT_TOT:(r + 1) * NT_TOT]
        tiles = fwd if (r % 2 == 0) else fwd[::-1]
        tile_of_node[ids] = tiles
        slot_of_node[ids] = r

    tile_sums = np.zeros(NT_TOT, np.int64)
    np.add.at(tile_sums, tile_of_node[:N_NODES], counts)

    # --- tiles -> cores: snake-deal in descending-edges order ---
    t_order = np.argsort(-tile_sums, kind="stable")
    core_of_tile = np.empty(NT_TOT, np.int32)
    cslot_of_tile = np.empty(NT_TOT, np.int32)  # per-core tile slot 0..TPC-1
    fwd8 = np.arange(N_CORES, dtype=np.int32)
    for r in range(TPC):
        ids = t_order[r * N_CORES:(r + 1) * N_CORES]
        cores = fwd8 if (r % 2 == 0) else fwd8[::-1]
        core_of_tile[ids] = cores
        cslot_of_tile[ids] = r

    # edges per (core, slot, table-half): src < H goes to half 0
    e_tile = tile_of_node[dst]
    e_core = core_of_tile[e_tile].astype(np.int64)
    e_cslot = cslot_of_tile[e_tile].astype(np.int64)
    e_half = (src >= H).astype(np.int64)
    ecnt = np.zeros((N_CORES, TPC, 2), np.int64)
    np.add.at(ecnt, (e_core, e_cslot, e_half), 1)

    # uniform per-slot chunk schedule (max over cores), per table half
    V = ecnt.max(axis=0)             # [TPC, 2] max valid rows per gather call
    KH = -(-V // P)                  # [TPC, 2] ceil div
    K = KH.sum(axis=1)               # combined chunks per slot
    g0 = np.concatenate([[0], np.cumsum(K)])
    CT = int(g0[-1])

    # --- per-core edge arrays laid out [P, CT] (partition = pos in chunk) ---
    esrc = np.zeros((N_CORES, P, CT), np.int32)
    edst = np.full((N_CORES, P, CT), -1.0, np.float32)
    erec = np.zeros((N_CORES, P, CT), np.float32)
    # int16 gather indices, wrapped [j%16, j//16] per gather block and
    # replicated over partition groups of 16 (dma_gather's index layout)
    eidx = np.zeros((N_CORES, P, 8 * CT), np.int16)

    ekey = (e_core * TPC + e_cslot) * 2 + e_half
    eorder = np.argsort(ekey, kind="stable")
    s_src = src[eorder]
    s_dst = dst[eorder]
    s_key = ekey[eorder]
    recip_all = 1.0 / np.maximum(cpad, 1).astype(np.float32)
    bounds = np.searchsorted(s_key, np.arange(N_CORES * TPC * 2 + 1))
    for c in range(N_CORES):
        for j in range(TPC):
            for h in range(2):
                key = (c * TPC + j) * 2 + h
                lo, hi = bounds[key], bounds[key + 1]
                n = hi - lo
                kh = int(KH[j, h])
                base = int(g0[j]) + (0 if h == 0 else int(KH[j, 0]))
                if n:
                    ss = s_src[lo:hi]
                    sd = s_dst[lo:hi]
                    pos = np.arange(n)
                    pp = pos % P
                    gg = base + pos // P
                    esrc[c, pp, gg] = ss
                    edst[c, pp, gg] = slot_of_node[sd].astype(np.float32)
                    erec[c, pp, gg] = recip_all[sd]
                if kh:
                    blk = np.zeros((16, kh * 8), np.int16)
                    if n:
                        val = (ss if h == 0 else ss - H).astype(np.int16)
                        blk[pos % 16, pos // 16] = val
                    eidx[c, :, 8 * base:8 * (base + kh)] = np.tile(blk, (8, 1))

    # node id for (core, tileslot, nodeslot) — for xT layout + output unshard
    node_at = np.empty((N_CORES, TPC, P), np.int64)
    node_ids = np.arange(NPAD)
    flat_idx = (core_of_tile[tile_of_node].astype(np.int64) * TPC * P
                + cslot_of_tile[tile_of_node].astype(np.int64) * P
                + slot_of_node)
    node_at.reshape(-1)[flat_idx] = node_ids
    return esrc, edst, erec, eidx, node_at, K, KH, g0, CT


def _build_program(K, KH, g0, CT, repeats=1, opts=None):
    opts = opts or {}
    import concourse.bacc as bacc
    import concourse.bass as bass
    import concourse.tile as tile
    import concourse.mybir as mybir

    f32 = mybir.dt.float32
    bf16 = mybir.dt.bfloat16
    i32 = mybir.dt.int32
    i16 = mybir.dt.int16

    nc = bacc.Bacc("TRN2", target_bir_lowering=False, debug=False,
                   num_devices=N_CORES,
                   num_swdge_queues=opts.get("nq", 1))

    x_table = nc.dram_tensor("x_table", [N_NODES, D], bf16, kind="ExternalInput")
    xTl = nc.dram_tensor("xTl", [P, TPC * D], bf16, kind="ExternalInput")
    w12l = nc.dram_tensor("w12l", [P, DC * D], bf16, kind="ExternalInput")
    w2nl = nc.dram_tensor("w2nl", [P, DC * D], bf16, kind="ExternalInput")
    b12 = nc.dram_tensor("b12", [1, D], bf16, kind="ExternalInput")
    esrc = nc.dram_tensor("esrc", [P, CT], i32, kind="ExternalInput")
    edst = nc.dram_tensor("edst", [P, CT], f32, kind="ExternalInput")
    erec = nc.dram_tensor("erec", [P, CT], f32, kind="ExternalInput")
    eidx = nc.dram_tensor("eidx", [P, 8 * CT], i16, kind="ExternalInput")
    iota_in = nc.dram_tensor("iota_in", [P, P], f32, kind="ExternalInput")
    ident_in = nc.dram_tensor("ident_in", [P, P], f32, kind="ExternalInput")
    if opts.get("hosty"):
        y_table = nc.dram_tensor("y_table", [N_NODES, D], bf16,
                                 kind="ExternalInput")
    out = nc.dram_tensor("out", [TPC * P, D], f32, kind="ExternalOutput")

    with tile.TileContext(nc) as tc:
        with (
            tc.tile_pool(name="res", bufs=1) as res,
            tc.tile_pool(name="gpool", bufs=opts.get("g_bufs", 3)) as gpool,
            tc.tile_pool(name="spool", bufs=int(K.max()) + 4) as spool,
            tc.tile_pool(name="mpool", bufs=3) as mpool,
            tc.tile_pool(name="opool", bufs=3) as opool,
            tc.tile_pool(name="pmean", bufs=opts.get("pmean_bufs", 2),
                         space="PSUM") as pmean,
            tc.tile_pool(name="ptrp", bufs=2, space="PSUM") as ptrp,
            tc.tile_pool(name="pout", bufs=opts.get("pout_bufs", 2),
                         space="PSUM") as pout,
        ):
            # resident constants
            xTl_sb = res.tile([P, TPC * D], bf16)
            nc.sync.dma_start(out=xTl_sb[:], in_=xTl[:])
            w12_sb = res.tile([P, DC * D], bf16)
            nc.sync.dma_start(out=w12_sb[:], in_=w12l[:])
            w2n_sb = res.tile([P, DC * D], bf16)
            nc.sync.dma_start(out=w2n_sb[:], in_=w2nl[:])
            b12_sb = res.tile([1, D], bf16)
            nc.sync.dma_start(out=b12_sb[:], in_=b12[:])
            esrc_sb = res.tile([P, CT], i32)
            nc.sync.dma_start(out=esrc_sb[:], in_=esrc[:])
            edst_sb = res.tile([P, CT], f32)
            nc.sync.dma_start(out=edst_sb[:], in_=edst[:])
            erec_sb = res.tile([P, CT], f32)
            nc.sync.dma_start(out=erec_sb[:], in_=erec[:])
            eidx_sb = res.tile([P, 8 * CT], i16)
            nc.sync.dma_start(out=eidx_sb[:], in_=eidx[:])
            iota_sb = res.tile([P, P], f32)
            nc.sync.dma_start(out=iota_sb[:], in_=iota_in[:])
            ident_sb = res.tile([P, P], f32)
            nc.sync.dma_start(out=ident_sb[:], in_=ident_in[:])
            ones_sb = res.tile([1, P], bf16)
            nc.vector.memset(ones_sb[:], 1.0)

            # software-pipelined: emit segment-phase(t), then dense-phase(t-1)
            pending = None  # (meanT_sb tile, tile idx)
            rep_tiles = [t for _ in range(repeats) for t in range(TPC)]

            def emit_gather(G, t, kt, gbase, table):
                k0, k1 = int(KH[t, 0]), int(KH[t, 1])
                for h, kh, coff in ((0, k0, 0), (1, k1, k0)):
                    if kh == 0:
                        continue
                    tbl = table[0:H, :] if h == 0 else table[H:N_NODES, :]
                    nc.gpsimd.dma_gather(
                        out_ap=G[:, coff * D:(coff + kh) * D].rearrange(
                            "p (k d) -> p k d", d=D),
                        in_ap=tbl,
                        idxs_ap=eidx_sb[:, 8 * (gbase + coff):
                                        8 * (gbase + coff + kh)],
                        num_idxs=kh * P,
                        num_idxs_reg=kh * P,
                        elem_size=D,
                        queue_num=(2 * t + h) % opts.get("nq", 1))

            def emit_S(gidx):
                S = spool.tile([P, P], bf16)
                nc.vector.tensor_scalar(
                    out=S[:], in0=iota_sb[:],
                    scalar1=edst_sb[:, gidx:gidx + 1],
                    scalar2=erec_sb[:, gidx:gidx + 1],
                    op0=mybir.AluOpType.is_equal,
                    op1=mybir.AluOpType.mult)
                return S

            if opts.get("hosty"):
                # single-phase: segment matmuls accumulate -mean@W2 directly
                # into the output PSUM from gathered y=x@(-W2) rows, then the
                # dense x@W12 + b12 matmuls extend the same group.
                for t in rep_tiles:
                    kt = int(K[t])
                    gbase = int(g0[t])
                    G = gpool.tile([P, kt * D], bf16, tag="G")
                    emit_gather(G, t, kt, gbase, y_table)
                    Ss = [emit_S(gbase + g) for g in range(kt)]
                    po = pout.tile([P, D], f32)
                    for g in range(kt):
                        nc.tensor.matmul(
                            out=po[:], lhsT=Ss[g][:],
                            rhs=G[:, g * D:(g + 1) * D],
                            start=(g == 0), stop=False)
                    for c in range(DC):
                        nc.tensor.matmul(
                            out=po[:],
                            lhsT=xTl_sb[:, (t * DC + c) * P:
                                        (t * DC + c + 1) * P],
                            rhs=w12_sb[:, c * D:(c + 1) * D],
                            start=False, stop=False)
                    nc.tensor.matmul(out=po[:], lhsT=ones_sb[:, :],
                                     rhs=b12_sb[:, :], start=False, stop=True)
                    out_sb = opool.tile([P, D], f32)
                    nc.vector.tensor_copy(out=out_sb[:], in_=po[:])
                    if not opts.get("no_store"):
                        nc.sync.dma_start(out=out[t * P:(t + 1) * P, :],
                                          in_=out_sb[:])
                rep_tiles = []

            def dense_phase(meanT_sb, t):
                po = pout.tile([P, D], f32)
                for c in range(DC):
                    nc.tensor.matmul(
                        out=po[:],
                        lhsT=xTl_sb[:, (t * DC + c) * P:(t * DC + c + 1) * P],
                        rhs=w12_sb[:, c * D:(c + 1) * D],
                        start=(c == 0), stop=False)
                nc.tensor.matmul(out=po[:], lhsT=ones_sb[:, :],
                                 rhs=b12_sb[:, :], start=False, stop=False)
                for c in range(DC):
                    nc.tensor.matmul(
                        out=po[:],
                        lhsT=meanT_sb[:, c * P:(c + 1) * P],
                        rhs=w2n_sb[:, c * D:(c + 1) * D],
                        start=False, stop=(c == DC - 1))
                out_sb = opool.tile([P, D], f32)
                nc.vector.tensor_copy(out=out_sb[:], in_=po[:])
                if not opts.get("no_store"):
                    nc.sync.dma_start(out=out[t * P:(t + 1) * P, :],
                                      in_=out_sb[:])

            for t in rep_tiles:
                kt = int(K[t])
                gbase = int(g0[t])
                pm = (None if opts.get("no_seg")
                      else pmean.tile([P, D], f32))
                # gather x[src] rows for this tile's edges via the ext-isa
                # dma_gather (one instruction per table half; indices are
                # int16 so the 50k-row table is split at H).
                # NOTE: batching an indirect_dma_start with a [128, kt]
                # offset AP works in CoreSim but mis-lowers on HW — use
                # dma_gather or per-chunk [128, 1] indirect DMAs only.
                G = (None if opts.get("no_seg")
                     else gpool.tile([P, kt * D], bf16, tag="G"))
                if opts.get("no_seg"):
                    pass
                elif opts.get("no_gather"):
                    nc.vector.memset(G[:], 0.0)
                elif opts.get("indirect"):
                    for g in range(kt):
                        gidx = gbase + g
                        nc.gpsimd.indirect_dma_start(
                            out=G[:, g * D:(g + 1) * D], out_offset=None,
                            in_=x_table[:],
                            in_offset=bass.IndirectOffsetOnAxis(
                                ap=esrc_sb[:, gidx:gidx + 1], axis=0))
                else:
                    k0, k1 = int(KH[t, 0]), int(KH[t, 1])
                    for h, kh, coff in ((0, k0, 0), (1, k1, k0)):
                        if kh == 0:
                            continue
                        tbl = x_table[0:H, :] if h == 0 else x_table[H:N_NODES, :]
                        nc.gpsimd.dma_gather(
                            out_ap=G[:, coff * D:(coff + kh) * D].rearrange(
                                "p (k d) -> p k d", d=D),
                            in_ap=tbl,
                            idxs_ap=eidx_sb[:, 8 * (gbase + coff):
                                            8 * (gbase + coff + kh)],
                            num_idxs=kh * P,
                            num_idxs_reg=kh * P,
                            elem_size=D,
                            queue_num=(2 * t + h) % opts.get("nq", 1))
                meanT_sb = mpool.tile([P, D], bf16, tag="meanT")
                if opts.get("no_seg"):
                    nc.vector.memset(meanT_sb[:], 0.0)
                else:
                    Ss = []
                    for g in range(kt):
                        gidx = gbase + g
                        S = spool.tile([P, P], bf16)
                        nc.vector.tensor_scalar(
                            out=S[:], in0=iota_sb[:],
                            scalar1=edst_sb[:, gidx:gidx + 1],
                            scalar2=erec_sb[:, gidx:gidx + 1],
                            op0=mybir.AluOpType.is_equal,
                            op1=mybir.AluOpType.mult)
                        Ss.append(S)
                    if opts.get("segC"):
                        # mean in [node, din]: one wide matmul per chunk
                        # (stationary S reused for all 512 moving cols), then
                        # transpose via 4 SBUF->SBUF DMA transposes.
                        # (PE identity transposes are broken here: bf16 PSUM
                        # gives wrong data, f32 wedges the exec unit.)
                        for g in range(kt):
                            nc.tensor.matmul(
                                out=pm[:],
                                lhsT=Ss[g][:],
                                rhs=G[:, g * D:(g + 1) * D],
                                start=(g == 0), stop=(g == kt - 1))
                        mean_sb = mpool.tile([P, D], bf16, tag="mean_bf")
                        nc.scalar.activation(
                            out=mean_sb[:], in_=pm[:],
                            func=mybir.ActivationFunctionType.Copy)
                        for c in range(DC):
                            nc.sync.dma_start_transpose(
                                out=meanT_sb[:, c * P:(c + 1) * P],
                                in_=mean_sb[:, c * P:(c + 1) * P])
                    else:
                        # meanT accumulation [din, node]: one PSUM
                        # accumulation group per 128-col slice (groups in the
                        # same bank must not interleave)
                        for c in range(DC):
                            for g in range(kt):
                                nc.tensor.matmul(
                                    out=pm[:, c * P:(c + 1) * P],
                                    lhsT=G[:, g * D + c * P:g * D + (c + 1) * P],
                                    rhs=Ss[g][:],
                                    start=(g == 0), stop=(g == kt - 1))
                        nc.scalar.activation(
                            out=meanT_sb[:], in_=pm[:],
                            func=mybir.ActivationFunctionType.Copy)
                if pending is not None:
                    dense_phase(*pending)
                pending = (meanT_sb, t)
            if pending is not None:
                dense_phase(*pending)

    nc.compile()
    return nc


def _pack(x, src, dst, W1, b1, W2, b2):
    counts = np.bincount(dst, minlength=N_NODES)
    esrc, edst, erec, eidx, node_at, K, KH, g0, CT = _route(src, dst, counts)

    x_pad = np.zeros((NPAD, D), np.float32)
    x_pad[:N_NODES] = x
    bf = ml_dtypes.bfloat16
    x_table = x.astype(bf)
    y_table = (x @ (-W2)).astype(bf)   # for the hosty variant

    W12 = (W1 + W2).astype(np.float32)
    W2n = (-W2).astype(np.float32)
    # w layout: [:, c*D:(c+1)*D] = W[c*128:(c+1)*128, :]
    w12l = np.ascontiguousarray(
        W12.reshape(DC, P, D).transpose(1, 0, 2).reshape(P, DC * D)).astype(bf)
    w2nl = np.ascontiguousarray(
        W2n.reshape(DC, P, D).transpose(1, 0, 2).reshape(P, DC * D)).astype(bf)
    b12 = (b1 + b2).astype(np.float32).reshape(1, D).astype(bf)

    in_maps = []
    for c in range(N_CORES):
        xo = x_pad[node_at[c].reshape(-1)]                    # [TPC*P, D]
        # xTl[p, (t*DC+cc)*P + n] = xo[t*P+n, cc*P+p]
        xTl = np.ascontiguousarray(
            xo.reshape(TPC, P, DC, P).transpose(3, 0, 2, 1).reshape(P, TPC * D)
        ).astype(bf)
        in_maps.append({
            "x_table": x_table,
            "y_table": y_table,
            "xTl": xTl,
            "w12l": w12l,
            "w2nl": w2nl,
            "b12": b12,
            "esrc": np.ascontiguousarray(esrc[c]),
            "edst": np.ascontiguousarray(edst[c]),
            "erec": np.ascontiguousarray(erec[c]),
            "eidx": np.ascontiguousarray(eidx[c]),
            "iota_in": np.tile(np.arange(P, dtype=np.float32), (P, 1)),
            "ident_in": np.eye(P, dtype=np.float32),
        })
    return in_maps, node_at, counts, K, KH, g0, CT


def _unshard(results, node_at, counts, x):
    out_full = np.empty((NPAD, D), np.float32)
    for c in range(N_CORES):
        out_full[node_at[c].reshape(-1)] = results[c]["out"]
    out_full = out_full[:N_NODES]
    zero = counts == 0
    out_full[zero] = x[zero]
    return out_full


def pack_from_inputs(inp):
    return _pack(np.asarray(inp["x"], np.float32),
                 np.asarray(inp["src"]).astype(np.int64),
                 np.asarray(inp["dst"]).astype(np.int64),
                 np.asarray(inp["W1"], np.float32),
                 np.asarray(inp["b1"], np.float32),
                 np.asarray(inp["W2"], np.float32),
                 np.asarray(inp["b2"], np.float32))


def kernel(**inputs):
    x = np.asarray(inputs["x"], np.float32)
    src = np.asarray(inputs["src"]).astype(np.int64)
    dst = np.asarray(inputs["dst"]).astype(np.int64)
    W1 = np.asarray(inputs["W1"], np.float32)
    b1 = np.asarray(inputs["b1"], np.float32)
    W2 = np.asarray(inputs["W2"], np.float32)
    b2 = np.asarray(inputs["b2"], np.float32)

    in_maps, node_at, counts, K, KH, g0, CT = _pack(x, src, dst, W1, b1, W2, b2)
    nc = _build_program(K, KH, g0, CT)

    from concourse.bass_utils import run_bass_kernel_spmd
    res = run_bass_kernel_spmd(nc, in_maps, core_ids=list(range(N_CORES)))
    return _unshard(res.results, node_at, counts, x)

